# revision 45
# baseline (speedup 1.0000x reference)
"""Trainium2 Bass kernel for gated pair-bias attention (AlphaFold-style).

Reference computation per (b=1, n) row:
  q,k,v = proj(input_*) reshaped to [H=8, S=256, C=32]; q /= sqrt(32)
  a = softmax(q@k^T + (mask-1)*1e9 + bias)      # [H, Q, K]
  o = (a@v) * sigmoid(input_q@wg + bg)          # gated
  out = o @ wo + bo                             # [S, 128]

Sharding: dim 1 (N=256 rows) split across 8 cores, 32 rows/core.

v2 design (engine-balanced):
  - natural q^T/k^T projections [d, s] in 2 chunks of 128 dims;
    per-head logits via 32-row PE tiles at partition offsets 32*(h%4)
    (explicit tile_position); same-position heads share a PSUM bank.
  - logits flow through 4 phases (kc x head-pair); each phase:
    4 matmuls -> one 1024-row exp on Act (mask folded in via the
    per-partition activation bias) -> bias multiply er*exp(bias)^T on
    DVE (bf16 2x) -> AV matmuls (kc-accumulated, Z column = ones).
  - gate z^T = wg^T x computed transposed [d, q]; bg added via a
    ones-matmul; tanh(0.5(z+bg)) on Act; sigma' = t+1 on DVE; the 0.5
    gate scale is folded into wo (host pre-scaled by 0.5).
  - AV natural [q, (h,33)] with Z in col 32 (ones column prefilled in
    the va pool buffers once); og1 = o * (1/Z) on DVE; PE transpose;
    gate-multiply fused into the transpose-PSUM read; final matmul
    with bo via ones-matmul; bf16 output DMA.
  - software pipelining: row n+1's projections/copies are emitted in
    the gaps between row n's logits phases so the PE fills the time
    the Act engine spends on exp; PSUM = 4 LT banks (2 tiles) +
    2 proj banks (2 one-bank tiles) + 2 AV banks.
"""

import math
import sys

sys.path.insert(0, "/opt/trn_rl_repo")

import numpy as np
import ml_dtypes

BF16 = ml_dtypes.bfloat16

B, N, S, CQ = 1, 256, 256, 128
H, C = 8, 32
NCORES = 8
NPER = N // NCORES  # 32 rows per core


def _build_bass(use_mask, use_bo=True):
    import concourse.bass as bass
    import concourse.bacc as bacc
    import concourse.tile as tile
    from concourse import mybir
    from concourse.masks import make_identity

    dt = mybir.dt
    AF = mybir.ActivationFunctionType
    ALU = mybir.AluOpType

    nc = bacc.Bacc()

    # ---- DRAM parameters (per-core shapes) ----
    x_all = nc.declare_dram_parameter("x_all", [NPER, 3, CQ, S], dt.bfloat16, isOutput=False)
    maskadd = nc.declare_dram_parameter("maskadd", [CQ, 2 * NPER], dt.float32, isOutput=False)
    ebT = nc.declare_dram_parameter("ebT", [2, CQ, H, S], dt.bfloat16, isOutput=False)
    wq2 = nc.declare_dram_parameter("wq2", [CQ, 2 * CQ], dt.bfloat16, isOutput=False)
    wk2 = nc.declare_dram_parameter("wk2", [CQ, 2 * CQ], dt.bfloat16, isOutput=False)
    wv2 = nc.declare_dram_parameter("wv2", [CQ, H * C], dt.bfloat16, isOutput=False)
    wg2 = nc.declare_dram_parameter("wg2", [CQ, 2 * CQ], dt.bfloat16, isOutput=False)
    wo05 = nc.declare_dram_parameter("wo05", [CQ, 2 * CQ], dt.bfloat16, isOutput=False)
    bgT = nc.declare_dram_parameter("bgT", [CQ, 2], dt.float32, isOutput=False)
    bo_p = nc.declare_dram_parameter("bo_p", [1, CQ], dt.bfloat16, isOutput=False)
    out_d = nc.declare_dram_parameter("out", [NPER, S, CQ], dt.bfloat16, isOutput=True)

    with tile.TileContext(nc) as tc:
        with (
            tc.tile_pool(name="const", bufs=1) as const,
            tc.tile_pool(name="xp", bufs=2) as xp,
            tc.tile_pool(name="qk", bufs=2) as qkp,
            tc.tile_pool(name="vap", bufs=3) as vap,
            tc.tile_pool(name="sgp", bufs=3) as sgp,
            tc.tile_pool(name="erp", bufs=2) as erp,
            tc.tile_pool(name="efp", bufs=4) as efp,
            tc.tile_pool(name="ogp", bufs=2) as ogp,
            tc.tile_pool(name="zp", bufs=2) as zp,
            tc.tile_pool(name="outp", bufs=2) as outp,
            tc.tile_pool(name="proj", bufs=2, space="PSUM") as projp,
            tc.tile_pool(name="lt", bufs=2, space="PSUM") as ltp,
            tc.tile_pool(name="av", bufs=1, space="PSUM") as avp,
        ):
            # ---------- once-per-core setup ----------
            wq_t = const.tile([CQ, 2, CQ], dt.bfloat16, tag="wq")
            wk_t = const.tile([CQ, 2, CQ], dt.bfloat16, tag="wk")
            wv_t = const.tile([CQ, H * C], dt.bfloat16, tag="wv")
            wg_t = const.tile([CQ, 2, CQ], dt.bfloat16, tag="wg")
            wo_t = const.tile([CQ, 2, CQ], dt.bfloat16, tag="wo")
            bg_t = const.tile([CQ, 2], dt.float32, tag="bg")
            bo_t = const.tile([1, CQ], dt.bfloat16, tag="bo")
            eb = const.tile([CQ, 2, H, S], dt.bfloat16, tag="eb")
            madd = const.tile([CQ, 2 * NPER], dt.float32, tag="madd")
            ones_r = const.tile([1, S], dt.bfloat16, tag="ones")
            ident = const.tile([CQ, CQ], dt.bfloat16, tag="ident")

            nc.sync.dma_start(out=wq_t, in_=wq2.ap().rearrange("p (a m) -> p a m", a=2))
            nc.sync.dma_start(out=wk_t, in_=wk2.ap().rearrange("p (a m) -> p a m", a=2))
            nc.sync.dma_start(out=wv_t, in_=wv2[:, :])
            nc.sync.dma_start(out=wg_t, in_=wg2.ap().rearrange("p (a m) -> p a m", a=2))
            nc.sync.dma_start(out=wo_t, in_=wo05.ap().rearrange("p (a m) -> p a m", a=2))
            nc.sync.dma_start(out=bg_t, in_=bgT[:, :])
            nc.sync.dma_start(out=bo_t, in_=bo_p[:, :])
            nc.sync.dma_start(out=eb, in_=ebT.ap().rearrange("a p h s -> p a h s"))
            nc.sync.dma_start(out=madd, in_=maskadd[:, :])
            nc.vector.memset(ones_r, 1.0)
            make_identity(nc, ident)

            # va pool: prefill the Z ones-column in all buffers once
            for _ in range(3):
                va_pre = vap.tile([CQ, 2, H, 33], dt.bfloat16, tag="va")
                nc.vector.memset(va_pre[:, :, :, 32], 1.0)

            # ---------- emission helpers ----------
            # gap chunks of the projection work for a row (dma already issued)
            def gap_q(xt):
                ps_q = projp.tile([CQ, 2, S], dt.float32, tag="proj")
                for dc in range(2):
                    nc.tensor.matmul(ps_q[:, dc], wq_t[:, dc], xt[:, 0],
                                     start=True, stop=True)
                qt_sb = qkp.tile([CQ, 2, S], dt.bfloat16, tag="qt")
                nc.vector.tensor_copy(out=qt_sb, in_=ps_q)
                return qt_sb

            def gap_k(xt):
                ps_k = projp.tile([CQ, 2, S], dt.float32, tag="proj")
                for dc in range(2):
                    nc.tensor.matmul(ps_k[:, dc], wk_t[:, dc], xt[:, 1],
                                     start=True, stop=True)
                kt_sb = qkp.tile([CQ, 2, S], dt.bfloat16, tag="kt")
                nc.vector.tensor_copy(out=kt_sb, in_=ps_k)
                return kt_sb

            def gap_v(xt):
                ps_v = projp.tile([CQ, 2, S], dt.float32, tag="proj")
                for c in range(2):
                    nc.tensor.matmul(ps_v[:, c], xt[:, 2, c * CQ:(c + 1) * CQ],
                                     wv_t, start=True, stop=True)
                va = vap.tile([CQ, 2, H, 33], dt.bfloat16, tag="va")
                nc.vector.tensor_copy(
                    out=va[:, :, :, 0:32],
                    in_=ps_v.rearrange("q kc (h x) -> q kc h x", x=32),
                )
                return va

            def gap_z_mm(xt):
                ps_z = projp.tile([CQ, 2, S], dt.float32, tag="proj")
                for c in range(2):
                    nc.tensor.matmul(ps_z[:, c], wg_t[:, c], xt[:, 0],
                                     start=True, stop=True)
                return ps_z

            def gap_z_act(ps_z):
                # sigma' = tanh(0.5 z + 0.5 bg) + 1 ; the x0.5 is in wo
                sgt = sgp.tile([CQ, 2, S], dt.bfloat16, tag="sgt")
                for dc in range(2):
                    nc.scalar.activation(sgt[:, dc], ps_z[:, dc], AF.Tanh,
                                         scale=0.5,
                                         bias=bg_t[:, dc:dc + 1])
                sg1 = sgp.tile([CQ, 2, S], dt.bfloat16, tag="sg1")
                # on gpsimd: keeps the (port-contended) DVE free
                nc.gpsimd.tensor_scalar(out=sg1, in0=sgt, scalar1=1.0,
                                        scalar2=1.0, op0=ALU.mult, op1=ALU.add)
                return sg1

            # logits phase t=(kc, pair p): 4 matmuls -> exp -> ef mul
            def lt_phase(n, kc, p, qt_sb, kt_sb, ef, mul_eng):
                ltt = ltp.tile([CQ, 2, 2 * S], dt.float32, tag="lt")
                for dc in range(2):
                    for b in range(2):  # alternate target banks
                        hh = 2 * p + b
                        off = 32 * hh
                        nc.tensor.matmul(
                            ltt[:, b, dc * S:(dc + 1) * S],
                            kt_sb[off:off + 32, dc, kc * CQ:(kc + 1) * CQ],
                            qt_sb[off:off + 32, dc, :],
                            start=True, stop=True,
                            tile_position=(off, 0),
                        )
                er = erp.tile([CQ, 2, 2, S], dt.bfloat16, tag="er")
                if use_mask:
                    nc.scalar.activation(
                        er, ltt.rearrange("q b (dc s) -> q b dc s", dc=2),
                        AF.Exp,
                        bias=madd[:, kc * NPER + n: kc * NPER + n + 1],
                    )
                else:
                    nc.scalar.activation(
                        er, ltt.rearrange("q b (dc s) -> q b dc s", dc=2),
                        AF.Exp,
                    )
                # ef[:, h, :] = er * exp(bias)^T for h = dc*4 + 2p + b
                h0 = 2 * p
                ef_view = bass.AP(
                    tensor=ef.tensor, offset=ef.offset + h0 * S,
                    ap=[list(ef.ap[0]), [4 * S, 2], [S, 2], [1, S]],
                )  # [q, dc, b, s]
                eb_view = bass.AP(
                    tensor=eb.tensor, offset=eb.offset + kc * H * S + h0 * S,
                    ap=[list(eb.ap[0]), [4 * S, 2], [S, 2], [1, S]],
                )
                mul_eng.tensor_mul(
                    ef_view, er.rearrange("q b dc s -> q dc b s"), eb_view)

            # AV matmuls for a head group; kc innermost so each PSUM
            # region's accumulation group is start->stop adjacent
            def av_chunk(heads, ef_pair, va, av_t):
                for h in heads:
                    for qc in range(2):
                        for kc in range(2):
                            nc.tensor.matmul(
                                av_t[:, qc, h * 33:(h + 1) * 33],
                                ef_pair[kc][:, h, qc * CQ:(qc + 1) * CQ],
                                va[:, kc, h, :],
                                start=(kc == 0), stop=(kc == 1),
                            )

            def emit_tail1(av_t):
                av_r = av_t[:, :, 0:H * 33].rearrange("q c (h x) -> q c h x", x=33)
                rz = zp.tile([CQ, 2, H], dt.float32, tag="rz")
                nc.vector.reciprocal(out=rz, in_=av_r[:, :, :, 32])
                og1 = ogp.tile([CQ, 2, H, 32], dt.bfloat16, tag="og1")
                nc.vector.tensor_mul(
                    og1, av_r[:, :, :, 0:32],
                    rz.unsqueeze(3).broadcast_to((CQ, 2, H, 32)))
                return og1

            def emit_tail2(n, sg1, og1):
                og1f = og1.rearrange("q c h x -> q c (h x)")
                ogT = []
                for dc in range(2):
                    pt = projp.tile([CQ, 2, CQ], dt.bfloat16, tag="proj")
                    for qc in range(2):
                        nc.tensor.transpose(
                            pt[:, qc], og1f[:, qc, dc * CQ:(dc + 1) * CQ], ident)
                    ot = ogp.tile([CQ, 2, CQ], dt.bfloat16, tag="ogT")
                    nc.vector.tensor_mul(
                        ot, pt, sg1[:, dc].rearrange("p (c q) -> p c q", c=2))
                    ogT.append(ot)
                ps_o = projp.tile([CQ, 2, CQ], dt.float32, tag="proj")
                for qc in range(2):
                    if use_bo:
                        nc.tensor.matmul(ps_o[:, qc], ogT[0][:, qc], wo_t[:, 0],
                                         start=True, stop=False)
                        nc.tensor.matmul(ps_o[:, qc], ogT[1][:, qc], wo_t[:, 1],
                                         start=False, stop=False)
                        nc.tensor.matmul(ps_o[:, qc], ones_r[:, 0:CQ], bo_t,
                                         start=False, stop=True)
                    else:
                        nc.tensor.matmul(ps_o[:, qc], ogT[0][:, qc], wo_t[:, 0],
                                         start=True, stop=False)
                        nc.tensor.matmul(ps_o[:, qc], ogT[1][:, qc], wo_t[:, 1],
                                         start=False, stop=True)
                out_sb = outp.tile([CQ, 2, CQ], dt.bfloat16, tag="osb")
                nc.scalar.copy(out=out_sb, in_=ps_o)
                nc.sync.dma_start(
                    out=out_d[n].rearrange("(qc p) c -> p qc c", p=CQ),
                    in_=out_sb)

            # ---------- 2-deep software-pipelined row loop ----------
            # body m: logits phases for row m interleaved with row m+1's
            # projections and row m-1's AV + tail.
            xt0 = xp.tile([CQ, 3, S], dt.bfloat16, tag="xt")
            nc.sync.dma_start(out=xt0, in_=x_all[0].rearrange("t p s -> p t s"))
            rows = {0: {"qt": gap_q(xt0), "kt": gap_k(xt0),
                        "va": gap_v(xt0), "sg": gap_z_act(gap_z_mm(xt0))}}

            PHASES = [(0, 0), (0, 1), (1, 0), (1, 1)]
            for m in range(NPER + 1):
                cur = rows.get(m)
                prev = rows.get(m - 1)
                if prev is not None:
                    prev["av"] = avp.tile([CQ, 2, 2 * S], dt.float32, tag="av",
                                          name="av_t")
                if cur is not None:
                    cur["ef"] = [efp.tile([CQ, H, S], dt.bfloat16, tag="ef",
                                          name=f"ef{kc}") for kc in range(2)]
                    if m + 1 < NPER:
                        xt = xp.tile([CQ, 3, S], dt.bfloat16, tag="xt")
                        nc.sync.dma_start(
                            out=xt, in_=x_all[m + 1].rearrange("t p s -> p t s"))
                        rows[m + 1] = nxt = {}

                    # kc=0 logits phases (32-row mode), ef-muls on gpsimd
                    for t in (0, 1):
                        kc, p = PHASES[t]
                        lt_phase(m, kc, p, cur["qt"], cur["kt"], cur["ef"][kc],
                                 nc.gpsimd)
                # 128-mode block: AV for the previous row interleaved with
                # next row's projections (long streams hide the AV LDWs)
                nxt_proj = cur is not None and m + 1 < NPER
                if prev is not None:
                    av_chunk((0, 1), prev["ef"], prev["va"], prev["av"])
                if nxt_proj:
                    nxt["qt"] = gap_q(xt)
                if prev is not None:
                    av_chunk((2, 3), prev["ef"], prev["va"], prev["av"])
                if nxt_proj:
                    nxt["kt"] = gap_k(xt)
                if prev is not None:
                    av_chunk((4, 5), prev["ef"], prev["va"], prev["av"])
                if nxt_proj:
                    nxt["va"] = gap_v(xt)
                if prev is not None:
                    av_chunk((6, 7), prev["ef"], prev["va"], prev["av"])
                if nxt_proj:
                    ps_z = gap_z_mm(xt)
                if cur is not None:
                    # kc=1 logits phases, ef-muls on DVE
                    for t in (2, 3):
                        kc, p = PHASES[t]
                        lt_phase(m, kc, p, cur["qt"], cur["kt"], cur["ef"][kc],
                                 nc.vector)
                if prev is not None:
                    og1 = emit_tail1(prev["av"])
                if nxt_proj:
                    nxt["sg"] = gap_z_act(ps_z)
                if prev is not None:
                    emit_tail2(m - 1, prev["sg"], og1)
                    del rows[m - 1]
    if not nc.is_finalized():
        nc.finalize()
    return nc


_NC_CACHE = {}


def _get_nc(use_mask, use_bo):
    key = (use_mask, use_bo)
    if key not in _NC_CACHE:
        _NC_CACHE[key] = _build_bass(use_mask, use_bo)
    return _NC_CACHE[key]


def kernel(input_q, input_k, input_v, mask, bias, wq, wk, wv, wg, bg, wo, bo):
    from concourse.bass_utils import run_bass_kernel_spmd

    use_mask = not bool(np.all(np.asarray(mask) == 1.0))
    use_bo = bool(np.any(np.asarray(bo) != 0.0))
    nc = _get_nc(use_mask, use_bo)

    # ---- host-side input prep (sharding + layout) ----
    wq_s = (wq / math.sqrt(C)).astype(np.float32)
    wq_b = wq_s.astype(BF16)                      # [128, 256] natural chunks
    wk_b = wk.astype(BF16)
    wv_b = wv.astype(BF16)
    wg_b = wg.astype(BF16)
    wo_b = (wo.astype(np.float32) * 0.5).astype(BF16)   # [256, 128]
    # wo05 layout [128, 2, 128]: wo05[p, dc, c] = 0.5*wo[dc*128+p, c]
    wo_t = np.ascontiguousarray(
        wo_b.reshape(2, CQ, CQ).transpose(1, 0, 2).reshape(CQ, 2 * CQ))
    # bgT[p, dc] = 0.5*bg[dc*128+p] (tanh bias, f32)
    bg_T = np.ascontiguousarray(
        (0.5 * bg.astype(np.float32)).reshape(2, CQ).T)
    bo_b = bo.reshape(1, CQ).astype(BF16)
    # exp(bias)^T [kc, 128, h, q]
    ebv = np.exp(bias[0, 0].astype(np.float32))          # [H, Q, K]
    ebT = np.ascontiguousarray(
        ebv.transpose(2, 0, 1).reshape(2, CQ, H, S)).astype(BF16)

    in_maps = []
    for i in range(NCORES):
        n0 = i * NPER
        sl = slice(n0, n0 + NPER)
        xq = input_q[0, sl].transpose(0, 2, 1)      # [NPER, 128, 256]
        xk = input_k[0, sl].transpose(0, 2, 1)
        xv = input_v[0, sl].transpose(0, 2, 1)
        x_all = np.ascontiguousarray(
            np.stack([xq, xk, xv], axis=1)).astype(BF16)  # [NPER,3,128,256]
        m = mask[0, sl, 0, 0, :].astype(np.float64)  # [NPER, 256]
        madd = np.maximum((m - 1.0) * 1.0e9, -60000.0).astype(np.float32)
        # maskadd[p, kc*NPER + n] = madd[n, kc*128 + p]
        mT = np.ascontiguousarray(
            madd.T.reshape(2, CQ, NPER).transpose(1, 0, 2).reshape(CQ, 2 * NPER)
        ).astype(np.float32)
        in_maps.append({
            "x_all": x_all, "maskadd": mT, "ebT": ebT,
            "wq2": wq_b, "wk2": wk_b, "wv2": wv_b, "wg2": wg_b,
            "wo05": wo_t, "bgT": bg_T, "bo_p": bo_b,
        })

    res = run_bass_kernel_spmd(nc, in_maps, list(range(NCORES)))
    out = np.concatenate([r["out"][None] for r in res.results], axis=0)
    return out.reshape(1, N, S, CQ).astype(np.float32)


if __name__ == "__main__":
    rng = np.random.default_rng(0)
    inps = {
        "input_q": rng.standard_normal((B, N, S, CQ), dtype=np.float32),
        "input_k": rng.standard_normal((B, N, S, CQ), dtype=np.float32),
        "input_v": rng.standard_normal((B, N, S, CQ), dtype=np.float32),
        "mask": np.ones((B, N, 1, 1, S), dtype=np.float32),
        "bias": rng.standard_normal((B, 1, H, S, S), dtype=np.float32),
        "wq": rng.standard_normal((CQ, H * C), dtype=np.float32) * 0.05,
        "wk": rng.standard_normal((CQ, H * C), dtype=np.float32) * 0.05,
        "wv": rng.standard_normal((CQ, H * C), dtype=np.float32) * 0.05,
        "wg": rng.standard_normal((CQ, H * C), dtype=np.float32) * 0.05,
        "bg": np.ones((H * C,), dtype=np.float32),
        "wo": rng.standard_normal((H * C, CQ), dtype=np.float32) * 0.05,
        "bo": np.zeros((CQ,), dtype=np.float32),
    }
    out = kernel(**inps)
    print("out shape", out.shape, out.dtype, float(np.abs(out).mean()))
